# Initial kernel scaffold
#
"""Trainium2 Bass kernel for nn_CrossAttention (B=4, N=M=2048, 8 heads x 64).

Sharding: 8 cores = batch(4) x query-half(2). Core c handles batch c//2,
query rows [(c%2)*1024, (c%2+1)*1024). Context/weights replicated per batch
pair; no cross-core communication.

v2 design (vs baseline at 266us):
- Heads processed in PAIRS packed on SBUF partition halves: even head of a
  pair lives on partitions 0-63, odd head on 64-127. The two q@k sim matmuls
  (contraction = DIM_HEAD = 64) then run CONCURRENTLY as 64x128 PE row-tiles
  T0/T8 (tile_position auto-derived from base partitions 0/64), doubling sim
  throughput vs the half-idle 128x128 array.
- softmax exp is split across TWO engines: Scalar(ACT) table exp for ~9/17
  key-tiles, and a custom 2-op DVE exp (cubic ^32, inlined below) for 8/17.
  exp was the baseline's hidden bottleneck (136 x [128,1024] tiles ~ 156us on
  ACT alone).
- Out-projection runs at contraction 128 (head-pair packed avT layout +
  Wout rows hp*128..hp*128+127), halving its PE time.
- All big inputs are pre-cast to bf16 on HOST and DMA'd directly into
  persistent SBUF tiles (no on-chip f32->bf16 staging copies; half the DMA).

Per-core compute (all matmuls bf16, f32 PSUM):
  qT2[128,hp,1024] = tanh(Wq_hp^T @ xT)      (psum partition j=h*64+d maps
  kT2[128,hp,2048] = tanh(Wkvk_hp^T @ ctxT)   even head to parts 0-63, odd
  v65[128,mt,h,65] = [ctx@Wkv_v | 1]          to 64-127 automatically)
  per (hp, ich in 0,1), per key-tile mt (16 real + 1 null):
    ps[:,0:512]   = kT2[0:64]^T  @ qT2[0:64]   (T0)   } concurrent row-tiles
    ps[:,512:1024]= kT2[64:128]^T@ qT2[64:128] (T8)   }
    ex = exp(ps * 1/8)  bf16   -- ACT or DVE path, alternating
    avt_A += v65[mt,hA]^T @ ex[:,0:512]   avt_B += v65[mt,hB]^T @ ex[:,512:]
  normalize: avT2[0:64,hp] = avt_A[0:64]/avt_A[64]; odd head via DMA shift
  outT = sum_hp Wout_hp^T @ avT2[:,hp] + bout    (contraction 128)
Softmax needs no max subtraction: q,k tanh-bounded so |sim/8| <= 8.
"""

import sys

import numpy as np

sys.path.insert(0, "/opt/trn_rl_repo")

B, N, M = 4, 2048, 2048
DIM = 512
HEADS, DIM_HEAD = 8, 64
INNER = HEADS * DIM_HEAD
NSH = N // 2
SCALE = DIM_HEAD ** -0.5
N_CORES = 8
KO = DIM // 128          # 4 contraction tiles of the model dim
MT = M // 128            # 16 key tiles (+1 null)
HP = HEADS // 2          # 4 head pairs
DVE_MTS = frozenset((2, 6, 10, 14))  # exp tiles routed to DVE (rest: ACT)

_COMPILED = {}
LAST_EXEC_TIME_NS = None

# ---- custom DVE exp (inlined; registered into concourse.dve_ops at build
# time). exp(t), |t|<=8.32, via r(u)^32, u=t/32, r cubic in factored form
# ((v+a)^2+b)*(v+c), v=alpha*u. Two DVE ops: EXPQ_ANT computes r^4 (8 ALU
# stages), SQ3_ANT cubes-squares to ^32. ~8e-4 rel err in f32. ----
_EXP_ALPHA = 0.5496968857081804
_EXP_A = 0.3894847077979361
_EXP_B = 0.9781544874995082
_EXP_C = 0.8850503605418246
_DVE = {}


def _dve_exp_ops():
    if _DVE:
        return _DVE
    from concourse.dve_ops import (
        OPS, _SUB_OPCODE_FOR_NAME, _CUSTOM_DVE_ROW_BASE, CUSTOM_DVE_SPECS,
        DveOp,
    )
    from concourse.dve_spec import (
        Spec, Src0, C0, C1, C2, C3, sq, lower, _has_src1, _spill_c3_to_src1,
    )
    from concourse.dve_uop import DveOpSpec

    def register(name, spec):
        for op in OPS:
            if op.name == name:
                return op
        row = _CUSTOM_DVE_ROW_BASE + len(OPS)
        assert row < 0x20
        _SUB_OPCODE_FOR_NAME[name] = row
        shas = {}
        for ver in ("v3", "v4"):
            shas[ver] = DveOpSpec(
                name=name, opcode=row, uops=lower(spec, ver=ver),
                rd1_en=_has_src1(spec)).sha(ver)
        op = DveOp(name, spec, subdim=False, uops_sha=shas)
        OPS.append(op)
        CUSTOM_DVE_SPECS[name] = spec
        return op

    def ref_expq(in0, in1, s0, s1, imm2):
        v = in0 * s0
        q = (np.square(v + s1) + imm2) * (v + in1[..., :1])
        return np.square(np.square(q))

    v = Src0 * C0
    body = _spill_c3_to_src1(sq(sq((sq(v + C1) + C2) * (v + C3))))
    _DVE["expq"] = register("EXPQ_ANT", Spec(body=body, reference=ref_expq))
    _DVE["sq3"] = register(
        "SQ3_ANT",
        Spec(body=sq(sq(sq(Src0))),
             reference=lambda in0, s0, s1, imm2:
             np.square(np.square(np.square(in0)))))
    return _DVE


def _emit_dve_exp(nc, out_ap, in_ap, scr_ap, cc_ap, logical_scale):
    """out = exp(in * logical_scale) via EXPQ_ANT -> scr, SQ3_ANT -> out.
    cc_ap: [P,1] f32 SBUF AP pre-filled with _EXP_C."""
    ops = _dve_exp_ops()
    nc.vector._custom_dve(
        ops["expq"], out=scr_ap, in0=in_ap, in1=cc_ap,
        s0=float(logical_scale * _EXP_ALPHA / 32.0), s1=float(_EXP_A),
        imm2=float(_EXP_B))
    nc.vector._custom_dve(ops["sq3"], out=out_ap, in0=scr_ap)


def _build(debug=False):
    import concourse.tile as tile
    from concourse import bacc, mybir

    F32 = mybir.dt.float32
    BF16 = mybir.dt.bfloat16
    Act = mybir.ActivationFunctionType

    nc = bacc.Bacc("TRN2", target_bir_lowering=False, debug=False,
                   num_devices=N_CORES)

    # host pre-transposes x/ctx to [DIM, rows] and pre-casts to bf16
    x_d = nc.dram_tensor("x", [DIM, NSH], BF16, kind="ExternalInput").ap()
    ctx_d = nc.dram_tensor("ctx", [DIM, M], BF16, kind="ExternalInput").ap()
    wq_d = nc.dram_tensor("wq", [DIM, INNER], BF16, kind="ExternalInput").ap()
    wkv_d = nc.dram_tensor("wkv", [DIM, 2 * INNER], BF16,
                           kind="ExternalInput").ap()
    nk_d = nc.dram_tensor("nullk", [128, 1], F32, kind="ExternalInput").ap()
    nv_d = nc.dram_tensor("nullv", [1, DIM_HEAD], F32, kind="ExternalInput").ap()
    wout_d = nc.dram_tensor("wout", [INNER, DIM], BF16, kind="ExternalInput").ap()
    bout_d = nc.dram_tensor("bout", [128, 4], F32, kind="ExternalInput").ap()
    out_d = nc.dram_tensor("out", [DIM, NSH], F32, kind="ExternalOutput").ap()
    if debug:
        dbg_q = nc.dram_tensor("dbg_q", [128, HP * NSH], BF16,
                               kind="ExternalOutput").ap()
        dbg_k = nc.dram_tensor("dbg_k", [128, HP * M], BF16,
                               kind="ExternalOutput").ap()
        dbg_v = nc.dram_tensor("dbg_v", [128, MT * HEADS * 65], BF16,
                               kind="ExternalOutput").ap()
        dbg_ex = nc.dram_tensor("dbg_ex", [128, 2 * 1024], BF16,
                                kind="ExternalOutput").ap()
        dbg_avt = nc.dram_tensor("dbg_avt", [65, 2 * 512], F32,
                                 kind="ExternalOutput").ap()
        dbg_av = nc.dram_tensor("dbg_av", [128, HP * NSH], BF16,
                                kind="ExternalOutput").ap()
        dbg_denb = nc.dram_tensor("dbg_denb", [64, 2 * 512], F32,
                                  kind="ExternalOutput").ap()

    with tile.TileContext(nc) as tc:
        with (
            tc.tile_pool(name="persist", bufs=1) as P,
            tc.tile_pool(name="stage", bufs=4) as ST,
            tc.tile_pool(name="exp", bufs=3) as EX,
            tc.tile_pool(name="scr", bufs=2) as SCR,
            tc.tile_pool(name="pse", bufs=1, space="PSUM") as PSE,
            tc.tile_pool(name="pso", bufs=1, space="PSUM") as PSO,
            tc.tile_pool(name="avae", bufs=1, space="PSUM") as PSAE,
            tc.tile_pool(name="avbe", bufs=1, space="PSUM") as PSBE,
            tc.tile_pool(name="avao", bufs=1, space="PSUM") as PSAO,
            tc.tile_pool(name="avbo", bufs=1, space="PSUM") as PSBO,
        ):
            # ---- persistent SBUF tensors ----
            xT = P.tile([128, KO, NSH], BF16, tag="xT")
            ctxT = P.tile([128, KO, M], BF16, tag="ctxT")
            wq_b = P.tile([128, KO, INNER], BF16, tag="wq")
            wkv_b = P.tile([128, KO, 2 * INNER], BF16, tag="wkv")
            wout_b = P.tile([128, HP, DIM], BF16, tag="wout")
            qT2 = P.tile([128, HP, NSH], BF16, tag="qT2")
            kT2 = P.tile([128, HP, M], BF16, tag="kT2")
            v65 = P.tile([128, MT, HEADS, 65], BF16, tag="v65")
            kT_null2 = P.tile([128, 128], BF16, tag="kTnull")
            v65_null = P.tile([128, 65], BF16, tag="v65null")
            avT2 = P.tile([128, HP, NSH], BF16, tag="avT2")
            bout_sb = P.tile([128, 4], F32, tag="bout")
            cc = P.tile([128, 1], F32, tag="cc")

            # ---- constants / null token ----
            nc.vector.memset(cc[:], _EXP_C)
            nk_st = ST.tile([128, 1], F32, tag="nk")
            nc.sync.dma_start(nk_st[:], nk_d[:])
            nc.vector.memset(kT_null2[:], 0.0)
            nc.scalar.activation(kT_null2[:, 0:1], nk_st[:], Act.Tanh)
            nv_st = ST.tile([1, DIM_HEAD], F32, tag="nv")
            nc.sync.dma_start(nv_st[:], nv_d[:])
            nc.vector.memset(v65_null[:], 0.0)
            nc.vector.tensor_copy(v65_null[0:1, 0:DIM_HEAD], nv_st[:])
            nc.vector.memset(v65_null[0:1, 64:65], 1.0)
            nc.vector.memset(v65[:, :, :, 64:65], 1.0)
            nc.sync.dma_start(bout_sb[:], bout_d[:])

            # ---- direct bf16 loads ----
            for ko in range(KO):
                nc.sync.dma_start(xT[:, ko, :],
                                  x_d[ko * 128:(ko + 1) * 128, :])
                nc.sync.dma_start(wq_b[:, ko, :],
                                  wq_d[ko * 128:(ko + 1) * 128, :])

            # ---- projection unit: [128,1024] psum (2 chunk groups) + tanh ----
            def proj_unit(pool, w_sb, w_off, srcT, dstT, hp, col):
                ps = pool.tile([128, 1024], F32, tag="ps")
                for half in range(2):
                    for kt in range(KO):
                        nc.tensor.matmul(
                            ps[:, half * 512:(half + 1) * 512],
                            lhsT=w_sb[:, kt,
                                      w_off + hp * 128:w_off + (hp + 1) * 128],
                            rhs=srcT[:, kt,
                                     col + half * 512:col + (half + 1) * 512],
                            start=(kt == 0), stop=(kt == KO - 1))
                nc.scalar.activation(dstT[:, hp, col:col + 1024], ps[:],
                                     Act.Tanh)

            def pair_proj_units(hp):
                """3 filler units producing qT2/kT2 for pair hp."""
                pl = (PSE, PSO, PSE) if hp % 2 == 0 else (PSO, PSE, PSO)
                return (
                    [lambda: proj_unit(pl[0], wq_b, 0, xT, qT2, hp, 0)] +
                    [lambda c=c, p=p: proj_unit(p, wkv_b, 0, ctxT, kT2, hp, c)
                     for c, p in ((0, pl[1]), (1024, pl[2]))])

            u0, u1 = pair_proj_units(0), pair_proj_units(1)
            u0[0]()
            u1[0]()

            for ko in range(KO):
                nc.sync.dma_start(ctxT[:, ko, 0:1024],
                                  ctx_d[ko * 128:(ko + 1) * 128, 0:1024])
                nc.sync.dma_start(wkv_b[:, ko, :],
                                  wkv_d[ko * 128:(ko + 1) * 128, :])

            vp_tog = [0]

            def v_proj(mt0):
                ps = (PSE, PSO)[vp_tog[0]].tile([128, 1024], F32, tag="ps")
                vp_tog[0] ^= 1
                for i, mt in enumerate((mt0, mt0 + 1)):
                    for kt in range(KO):
                        nc.tensor.matmul(
                            ps[:, i * 512:(i + 1) * 512],
                            lhsT=ctxT[:, kt, mt * 128:(mt + 1) * 128],
                            rhs=wkv_b[:, kt, INNER:2 * INNER],
                            start=(kt == 0), stop=(kt == KO - 1))
                for i, mt in enumerate((mt0, mt0 + 1)):
                    nc.vector.tensor_copy(
                        v65[:, mt, :, 0:DIM_HEAD],
                        ps[:, i * 512:(i + 1) * 512]
                        .rearrange("p (h d) -> p h d", d=DIM_HEAD))

            u0[1]()
            u1[1]()
            for mt0 in range(0, 8, 2):
                v_proj(mt0)

            for ko in range(KO):
                nc.sync.dma_start(ctxT[:, ko, 1024:2048],
                                  ctx_d[ko * 128:(ko + 1) * 128, 1024:2048])
            u0[2]()
            u1[2]()
            for mt0 in range(8, MT, 2):
                v_proj(mt0)

            for hp in range(HP):
                nc.sync.dma_start(wout_b[:, hp, :],
                                  wout_d[hp * 128:(hp + 1) * 128, :])

            # ---- attention: TWO head-pair streams co-scheduled per mt step.
            # Each stream has its own single-buffered sim psum; while one
            # stream's exp runs, the PE works the other stream — keeping the
            # PE dense enough to hold the HAM clock at 2.4 GHz. ----
            def norm_pair(hp, c0, avt_a, avt_b, dbg):
                # head A: avT2[0:64,hp,c0:] = avt_a[0:64]/avt_a[64]
                rA = ST.tile([65, 512], F32, tag="recip")
                nc.vector.tensor_copy(rA[64:65, :], avt_a[64:65, :])
                r0A = ST.tile([1, 512], F32, tag="r0")
                nc.gpsimd.dma_start(r0A[0:1, :], rA[64:65, :])
                denbA = ST.tile([64, 512], F32, tag="denb")
                nc.gpsimd.partition_broadcast(denbA[:], r0A[0:1, :])
                rdenbA = ST.tile([64, 512], F32, tag="rdenb")
                nc.vector.reciprocal_approx_fast(rdenbA[:], denbA[:])
                if dbg:
                    nc.sync.dma_start(dbg_denb[:, 0:512], rdenbA[:])
                nc.vector.tensor_mul(avT2[0:64, hp, c0:c0 + 512],
                                     avt_a[0:64, :], rdenbA[:])
                # head B: normalize at partitions 0-63, DMA-shift to 64-127
                rB = ST.tile([65, 512], F32, tag="recip")
                nc.vector.tensor_copy(rB[64:65, :], avt_b[64:65, :])
                r0B = ST.tile([1, 512], F32, tag="r0")
                nc.gpsimd.dma_start(r0B[0:1, :], rB[64:65, :])
                denbB = ST.tile([64, 512], F32, tag="denb")
                nc.gpsimd.partition_broadcast(denbB[:], r0B[0:1, :])
                rdenbB = ST.tile([64, 512], F32, tag="rdenb")
                nc.vector.reciprocal_approx_fast(rdenbB[:], denbB[:])
                if dbg:
                    nc.sync.dma_start(dbg_denb[:, 512:1024], rdenbB[:])
                tmpB = ST.tile([64, 512], BF16, tag="tmpb")
                nc.vector.tensor_mul(tmpB[:], avt_b[0:64, :], rdenbB[:])
                nc.gpsimd.dma_start(avT2[64:128, hp, c0:c0 + 512], tmpB[:])

            DVE_E, DVE_O = frozenset((2, 6, 10, 14)), frozenset((0, 4, 8, 12))
            for sp in range(2):
                prs = (2 * sp, 2 * sp + 1)
                fillers = (pair_proj_units(2) + pair_proj_units(3)
                           if sp == 0 else [])
                for ich in range(2):
                    c0 = ich * 512
                    avt = {
                        prs[0]: (PSAE.tile([65, 512], F32, tag="avta",
                                           name="avtae"),
                                 PSBE.tile([65, 512], F32, tag="avtb",
                                           name="avtbe")),
                        prs[1]: (PSAO.tile([65, 512], F32, tag="avta",
                                           name="avtao"),
                                 PSBO.tile([65, 512], F32, tag="avtb",
                                           name="avtbo")),
                    }
                    for mt in range(MT + 1):
                        for pr in prs:
                            hA, hB = 2 * pr, 2 * pr + 1
                            if mt < MT:
                                lhsA = kT2[0:64, pr, mt * 128:(mt + 1) * 128]
                                lhsB = kT2[64:128, pr,
                                           mt * 128:(mt + 1) * 128]
                                vA = v65[:, mt, hA, :]
                                vB = v65[:, mt, hB, :]
                            else:
                                lhsA = kT_null2[0:64, :]
                                lhsB = kT_null2[64:128, :]
                                vA = vB = v65_null[:]
                            pool = PSE if pr % 2 == 0 else PSO
                            ps = pool.tile([128, 1024], F32, tag="ps")
                            nc.tensor.matmul(ps[:, 0:512], lhsT=lhsA,
                                             rhs=qT2[0:64, pr, c0:c0 + 512],
                                             start=True, stop=True)
                            nc.tensor.matmul(ps[:, 512:1024], lhsT=lhsB,
                                             rhs=qT2[64:128, pr,
                                                     c0:c0 + 512],
                                             start=True, stop=True)
                            ex = EX.tile([128, 1024], BF16, tag="ex")
                            dve = (mt in DVE_E if pr % 2 == 0
                                   else mt in DVE_O)
                            if dve:
                                scr = SCR.tile([128, 1024], F32, tag="scr")
                                _emit_dve_exp(nc, ex[:], ps[:], scr[:],
                                              cc[:], SCALE)
                            else:
                                nc.scalar.activation(ex[:], ps[:], Act.Exp,
                                                     scale=SCALE)
                            if debug and pr == 0 and ich == 0 and mt < 2:
                                nc.sync.dma_start(
                                    dbg_ex[:, mt * 1024:(mt + 1) * 1024],
                                    ex[:])
                            avt_a, avt_b = avt[pr]
                            nc.tensor.matmul(avt_a[:], lhsT=vA,
                                             rhs=ex[:, 0:512],
                                             start=(mt == 0), stop=(mt == MT))
                            nc.tensor.matmul(avt_b[:], lhsT=vB,
                                             rhs=ex[:, 512:1024],
                                             start=(mt == 0), stop=(mt == MT))
                        if mt in (4, 9, 14) and fillers:
                            fillers.pop(0)()
                    if debug and sp == 0 and ich == 0:
                        avst = ST.tile([65, 1024], F32, tag="avst")
                        nc.vector.tensor_copy(avst[:, 0:512], avt[0][0][:])
                        nc.vector.tensor_copy(avst[:, 512:1024], avt[0][1][:])
                        nc.sync.dma_start(dbg_avt[:], avst[:])
                    for pr in prs:
                        avt_a, avt_b = avt[pr]
                        norm_pair(pr, c0, avt_a, avt_b,
                                  debug and pr == 0 and ich == 0)

            if debug:
                nc.sync.dma_start(
                    dbg_q[:], qT2[:].rearrange("p a b -> p (a b)"))
                nc.sync.dma_start(
                    dbg_k[:], kT2[:].rearrange("p a b -> p (a b)"))
                nc.sync.dma_start(
                    dbg_v[:], v65[:].rearrange("p a b c -> p (a b c)"))
                nc.sync.dma_start(
                    dbg_av[:], avT2[:].rearrange("p a b -> p (a b)"))

            # ---- out-projection at contraction 128 ----
            outT_d = out_d.rearrange("(co p) i -> p co i", p=128)
            for ct in range(4):
                ps_o = (PSE, PSO)[ct % 2].tile([128, 1024], F32, tag="ps")
                for ich in range(2):
                    for hp in range(HP):
                        nc.tensor.matmul(
                            ps_o[:, ich * 512:(ich + 1) * 512],
                            lhsT=wout_b[:, hp, ct * 128:(ct + 1) * 128],
                            rhs=avT2[:, hp, ich * 512:(ich + 1) * 512],
                            start=(hp == 0), stop=(hp == HP - 1))
                ost = ST.tile([128, 1024], F32, tag="ost")
                nc.vector.tensor_add(ost[:], ps_o[:],
                                     bout_sb[:, ct:ct + 1]
                                     .to_broadcast((128, 1024)))
                deng = nc.sync if ct % 2 == 0 else nc.scalar
                deng.dma_start(outT_d[:, ct, :], ost[:])

    nc.compile()
    return nc


def _get_compiled():
    if "nc" not in _COMPILED:
        _COMPILED["nc"] = _build()
    return _COMPILED["nc"]


def kernel(x, context, Wq, Wkv, null_k, null_v, Wout, bout):
    global LAST_EXEC_TIME_NS
    import ml_dtypes
    from concourse.bass_utils import run_bass_kernel_spmd

    BF = ml_dtypes.bfloat16
    x = np.asarray(x, dtype=np.float32)
    context = np.asarray(context, dtype=np.float32)
    nk2 = np.tile(np.asarray(null_k, np.float32).reshape(64, 1), (2, 1)).copy()
    nv = np.asarray(null_v, np.float32).reshape(1, 64)
    bout_r = np.asarray(bout, np.float32).reshape(4, 128).T.copy()
    wq = np.ascontiguousarray(np.asarray(Wq, np.float32)).astype(BF)
    wkv = np.ascontiguousarray(np.asarray(Wkv, np.float32)).astype(BF)
    wout = np.ascontiguousarray(np.asarray(Wout, np.float32)).astype(BF)

    in_maps = []
    ctxT_all = [np.ascontiguousarray(context[b].T).astype(BF)
                for b in range(B)]
    for c in range(N_CORES):
        b, j = c // 2, c % 2
        in_maps.append({
            "x": np.ascontiguousarray(
                x[b, j * NSH:(j + 1) * NSH, :].T).astype(BF),
            "ctx": ctxT_all[b],
            "wq": wq,
            "wkv": wkv,
            "nullk": nk2,
            "nullv": nv,
            "wout": wout,
            "bout": bout_r,
        })

    nc = _get_compiled()
    res = run_bass_kernel_spmd(nc, in_maps, core_ids=list(range(N_CORES)))
    LAST_EXEC_TIME_NS = res.exec_time_ns

    out = np.empty((B, N, DIM), np.float32)
    for c in range(N_CORES):
        b, j = c // 2, c % 2
        out[b, j * NSH:(j + 1) * NSH, :] = res.results[c]["out"].T
    return out



# revision 1
# speedup vs baseline: 1.0545x; 1.0545x over previous
"""Trainium2 Bass kernel for nn_CrossAttention (B=4, N=M=2048, 8 heads x 64).

Sharding: 8 cores = batch(4) x query-half(2). Core c handles batch c//2,
query rows [(c%2)*1024, (c%2+1)*1024). Context/weights replicated per batch
pair; no cross-core communication.

v2 design (vs baseline at 266us):
- Heads processed in PAIRS packed on SBUF partition halves: even head of a
  pair lives on partitions 0-63, odd head on 64-127. The two q@k sim matmuls
  (contraction = DIM_HEAD = 64) then run CONCURRENTLY as 64x128 PE row-tiles
  T0/T8 (tile_position auto-derived from base partitions 0/64), doubling sim
  throughput vs the half-idle 128x128 array.
- softmax exp is split across TWO engines: Scalar(ACT) table exp for ~9/17
  key-tiles, and a custom 2-op DVE exp (cubic ^32, inlined below) for 8/17.
  exp was the baseline's hidden bottleneck (136 x [128,1024] tiles ~ 156us on
  ACT alone).
- Out-projection runs at contraction 128 (head-pair packed avT layout +
  Wout rows hp*128..hp*128+127), halving its PE time.
- All big inputs are pre-cast to bf16 on HOST and DMA'd directly into
  persistent SBUF tiles (no on-chip f32->bf16 staging copies; half the DMA).

Per-core compute (all matmuls bf16, f32 PSUM):
  qT2[128,hp,1024] = tanh(Wq_hp^T @ xT)      (psum partition j=h*64+d maps
  kT2[128,hp,2048] = tanh(Wkvk_hp^T @ ctxT)   even head to parts 0-63, odd
  v65[128,mt,h,65] = [ctx@Wkv_v | 1]          to 64-127 automatically)
  per (hp, ich in 0,1), per key-tile mt (16 real + 1 null):
    ps[:,0:512]   = kT2[0:64]^T  @ qT2[0:64]   (T0)   } concurrent row-tiles
    ps[:,512:1024]= kT2[64:128]^T@ qT2[64:128] (T8)   }
    ex = exp(ps * 1/8)  bf16   -- ACT or DVE path, alternating
    avt_A += v65[mt,hA]^T @ ex[:,0:512]   avt_B += v65[mt,hB]^T @ ex[:,512:]
  normalize: avT2[0:64,hp] = avt_A[0:64]/avt_A[64]; odd head via DMA shift
  outT = sum_hp Wout_hp^T @ avT2[:,hp] + bout    (contraction 128)
Softmax needs no max subtraction: q,k tanh-bounded so |sim/8| <= 8.
"""

import sys

import numpy as np

sys.path.insert(0, "/opt/trn_rl_repo")

B, N, M = 4, 2048, 2048
DIM = 512
HEADS, DIM_HEAD = 8, 64
INNER = HEADS * DIM_HEAD
NSH = N // 2
SCALE = DIM_HEAD ** -0.5
N_CORES = 8
KO = DIM // 128          # 4 contraction tiles of the model dim
MT = M // 128            # 16 key tiles (+1 null)
HP = HEADS // 2          # 4 head pairs
DVE_MTS = frozenset((2, 6, 10, 14))  # exp tiles routed to DVE (rest: ACT)

_COMPILED = {}
LAST_EXEC_TIME_NS = None

# ---- custom DVE exp (inlined; registered into concourse.dve_ops at build
# time). exp(t), |t|<=8.32, via r(u)^32, u=t/32, r cubic in factored form
# ((v+a)^2+b)*(v+c), v=alpha*u. Two DVE ops: EXPQ_ANT computes r^4 (8 ALU
# stages), SQ3_ANT cubes-squares to ^32. ~8e-4 rel err in f32. ----
_EXP_ALPHA = 0.5496968857081804
_EXP_A = 0.3894847077979361
_EXP_B = 0.9781544874995082
_EXP_C = 0.8850503605418246
_DVE = {}


def _dve_exp_ops():
    if _DVE:
        return _DVE
    from concourse.dve_ops import (
        OPS, _SUB_OPCODE_FOR_NAME, _CUSTOM_DVE_ROW_BASE, CUSTOM_DVE_SPECS,
        DveOp,
    )
    from concourse.dve_spec import (
        Spec, Src0, C0, C1, C2, C3, sq, lower, _has_src1, _spill_c3_to_src1,
    )
    from concourse.dve_uop import DveOpSpec

    def register(name, spec):
        for op in OPS:
            if op.name == name:
                return op
        row = _CUSTOM_DVE_ROW_BASE + len(OPS)
        assert row < 0x20
        _SUB_OPCODE_FOR_NAME[name] = row
        shas = {}
        for ver in ("v3", "v4"):
            shas[ver] = DveOpSpec(
                name=name, opcode=row, uops=lower(spec, ver=ver),
                rd1_en=_has_src1(spec)).sha(ver)
        op = DveOp(name, spec, subdim=False, uops_sha=shas)
        OPS.append(op)
        CUSTOM_DVE_SPECS[name] = spec
        return op

    def ref_expq(in0, in1, s0, s1, imm2):
        v = in0 * s0
        q = (np.square(v + s1) + imm2) * (v + in1[..., :1])
        return np.square(np.square(q))

    v = Src0 * C0
    body = _spill_c3_to_src1(sq(sq((sq(v + C1) + C2) * (v + C3))))
    _DVE["expq"] = register("EXPQ_ANT", Spec(body=body, reference=ref_expq))
    _DVE["sq3"] = register(
        "SQ3_ANT",
        Spec(body=sq(sq(sq(Src0))),
             reference=lambda in0, s0, s1, imm2:
             np.square(np.square(np.square(in0)))))
    return _DVE


def _emit_dve_exp(nc, out_ap, in_ap, scr_ap, cc_ap, logical_scale):
    """out = exp(in * logical_scale) via EXPQ_ANT -> scr, SQ3_ANT -> out.
    cc_ap: [P,1] f32 SBUF AP pre-filled with _EXP_C."""
    ops = _dve_exp_ops()
    nc.vector._custom_dve(
        ops["expq"], out=scr_ap, in0=in_ap, in1=cc_ap,
        s0=float(logical_scale * _EXP_ALPHA / 32.0), s1=float(_EXP_A),
        imm2=float(_EXP_B))
    nc.vector._custom_dve(ops["sq3"], out=out_ap, in0=scr_ap)


def _build(debug=False):
    import concourse.tile as tile
    from concourse import bacc, mybir

    F32 = mybir.dt.float32
    BF16 = mybir.dt.bfloat16
    Act = mybir.ActivationFunctionType

    nc = bacc.Bacc("TRN2", target_bir_lowering=False, debug=False,
                   num_devices=N_CORES)

    # host pre-transposes x/ctx to [DIM, rows] and pre-casts to bf16
    x_d = nc.dram_tensor("x", [DIM, NSH], BF16, kind="ExternalInput").ap()
    ctx_d = nc.dram_tensor("ctx", [DIM, M], BF16, kind="ExternalInput").ap()
    wq_d = nc.dram_tensor("wq", [DIM, INNER], BF16, kind="ExternalInput").ap()
    wkv_d = nc.dram_tensor("wkv", [DIM, 2 * INNER], BF16,
                           kind="ExternalInput").ap()
    nk_d = nc.dram_tensor("nullk", [128, 1], F32, kind="ExternalInput").ap()
    nv_d = nc.dram_tensor("nullv", [1, DIM_HEAD], F32, kind="ExternalInput").ap()
    wout_d = nc.dram_tensor("wout", [INNER, DIM], BF16, kind="ExternalInput").ap()
    bout_d = nc.dram_tensor("bout", [128, 4], F32, kind="ExternalInput").ap()
    out_d = nc.dram_tensor("out", [DIM, NSH], F32, kind="ExternalOutput").ap()
    if debug:
        dbg_q = nc.dram_tensor("dbg_q", [128, HP * NSH], BF16,
                               kind="ExternalOutput").ap()
        dbg_k = nc.dram_tensor("dbg_k", [128, HP * M], BF16,
                               kind="ExternalOutput").ap()
        dbg_v = nc.dram_tensor("dbg_v", [128, MT * HEADS * 65], BF16,
                               kind="ExternalOutput").ap()
        dbg_ex = nc.dram_tensor("dbg_ex", [128, 2 * 1024], BF16,
                                kind="ExternalOutput").ap()
        dbg_avt = nc.dram_tensor("dbg_avt", [65, 2 * 512], F32,
                                 kind="ExternalOutput").ap()
        dbg_av = nc.dram_tensor("dbg_av", [128, HP * NSH], BF16,
                                kind="ExternalOutput").ap()
        dbg_denb = nc.dram_tensor("dbg_denb", [64, 2 * 512], F32,
                                  kind="ExternalOutput").ap()

    with tile.TileContext(nc) as tc:
        with (
            tc.tile_pool(name="persist", bufs=1) as P,
            tc.tile_pool(name="stage", bufs=4) as ST,
            tc.tile_pool(name="exp", bufs=3) as EX,
            tc.tile_pool(name="scr", bufs=2) as SCR,
            tc.tile_pool(name="pse", bufs=1, space="PSUM") as PSE,
            tc.tile_pool(name="pso", bufs=1, space="PSUM") as PSO,
            tc.tile_pool(name="avae", bufs=1, space="PSUM") as PSAE,
            tc.tile_pool(name="avbe", bufs=1, space="PSUM") as PSBE,
            tc.tile_pool(name="avao", bufs=1, space="PSUM") as PSAO,
            tc.tile_pool(name="avbo", bufs=1, space="PSUM") as PSBO,
        ):
            # ---- persistent SBUF tensors ----
            xT = P.tile([128, KO, NSH], BF16, tag="xT")
            ctxT = P.tile([128, KO, M], BF16, tag="ctxT")
            wq_b = P.tile([128, KO, INNER], BF16, tag="wq")
            wkv_b = P.tile([128, KO, 2 * INNER], BF16, tag="wkv")
            wout_b = P.tile([128, HP, DIM], BF16, tag="wout")
            qT2 = P.tile([128, HP, NSH], BF16, tag="qT2")
            kT2 = P.tile([128, HP, M], BF16, tag="kT2")
            v65 = P.tile([128, MT, HEADS, 65], BF16, tag="v65")
            kT_null2 = P.tile([128, 128], BF16, tag="kTnull")
            v65_null = P.tile([128, 65], BF16, tag="v65null")
            avT2 = P.tile([128, HP, NSH], BF16, tag="avT2")
            bout_sb = P.tile([128, 4], F32, tag="bout")
            cc = P.tile([128, 1], F32, tag="cc")

            # ---- constants / null token ----
            nc.vector.memset(cc[:], _EXP_C)
            nk_st = ST.tile([128, 1], F32, tag="nk")
            nc.sync.dma_start(nk_st[:], nk_d[:])
            nc.vector.memset(kT_null2[:], 0.0)
            nc.scalar.activation(kT_null2[:, 0:1], nk_st[:], Act.Tanh)
            nv_st = ST.tile([1, DIM_HEAD], F32, tag="nv")
            nc.sync.dma_start(nv_st[:], nv_d[:])
            nc.vector.memset(v65_null[:], 0.0)
            nc.vector.tensor_copy(v65_null[0:1, 0:DIM_HEAD], nv_st[:])
            nc.vector.memset(v65_null[0:1, 64:65], 1.0)
            nc.vector.memset(v65[:, :, :, 64:65], 1.0)
            nc.sync.dma_start(bout_sb[:], bout_d[:])

            # ---- direct bf16 loads ----
            for ko in range(KO):
                nc.sync.dma_start(xT[:, ko, :],
                                  x_d[ko * 128:(ko + 1) * 128, :])
                nc.sync.dma_start(wq_b[:, ko, :],
                                  wq_d[ko * 128:(ko + 1) * 128, :])

            # ---- projection unit: [128,1024] psum (2 chunk groups) + tanh ----
            def proj_unit(pool, w_sb, w_off, srcT, dstT, hp, col):
                ps = pool.tile([128, 1024], F32, tag="ps")
                for half in range(2):
                    for kt in range(KO):
                        nc.tensor.matmul(
                            ps[:, half * 512:(half + 1) * 512],
                            lhsT=w_sb[:, kt,
                                      w_off + hp * 128:w_off + (hp + 1) * 128],
                            rhs=srcT[:, kt,
                                     col + half * 512:col + (half + 1) * 512],
                            start=(kt == 0), stop=(kt == KO - 1))
                nc.scalar.activation(dstT[:, hp, col:col + 1024], ps[:],
                                     Act.Tanh)

            def pair_proj_units(hp):
                """3 filler units producing qT2/kT2 for pair hp."""
                pl = (PSE, PSO, PSE) if hp % 2 == 0 else (PSO, PSE, PSO)
                return (
                    [lambda: proj_unit(pl[0], wq_b, 0, xT, qT2, hp, 0)] +
                    [lambda c=c, p=p: proj_unit(p, wkv_b, 0, ctxT, kT2, hp, c)
                     for c, p in ((0, pl[1]), (1024, pl[2]))])

            u0, u1 = pair_proj_units(0), pair_proj_units(1)
            u0[0]()
            u1[0]()

            for ko in range(KO):
                nc.sync.dma_start(ctxT[:, ko, 0:1024],
                                  ctx_d[ko * 128:(ko + 1) * 128, 0:1024])
                nc.sync.dma_start(wkv_b[:, ko, :],
                                  wkv_d[ko * 128:(ko + 1) * 128, :])

            vp_tog = [0]

            def v_proj(mt0):
                ps = (PSE, PSO)[vp_tog[0]].tile([128, 1024], F32, tag="ps")
                vp_tog[0] ^= 1
                for i, mt in enumerate((mt0, mt0 + 1)):
                    for kt in range(KO):
                        nc.tensor.matmul(
                            ps[:, i * 512:(i + 1) * 512],
                            lhsT=ctxT[:, kt, mt * 128:(mt + 1) * 128],
                            rhs=wkv_b[:, kt, INNER:2 * INNER],
                            start=(kt == 0), stop=(kt == KO - 1))
                for i, mt in enumerate((mt0, mt0 + 1)):
                    nc.vector.tensor_copy(
                        v65[:, mt, :, 0:DIM_HEAD],
                        ps[:, i * 512:(i + 1) * 512]
                        .rearrange("p (h d) -> p h d", d=DIM_HEAD))

            u0[1]()
            u1[1]()
            for mt0 in range(0, 8, 2):
                v_proj(mt0)

            for ko in range(KO):
                nc.sync.dma_start(ctxT[:, ko, 1024:2048],
                                  ctx_d[ko * 128:(ko + 1) * 128, 1024:2048])
            u0[2]()
            u1[2]()
            for mt0 in range(8, MT, 2):
                v_proj(mt0)

            for hp in range(HP):
                nc.sync.dma_start(wout_b[:, hp, :],
                                  wout_d[hp * 128:(hp + 1) * 128, :])

            # ---- attention: TWO head-pair streams co-scheduled per mt step.
            # Each stream has its own single-buffered sim psum; while one
            # stream's exp runs, the PE works the other stream — keeping the
            # PE dense enough to hold the HAM clock at 2.4 GHz. ----
            def norm_pair(hp, c0, avt_a, avt_b, dbg):
                # head A: avT2[0:64,hp,c0:] = avt_a[0:64]/avt_a[64]
                rA = ST.tile([65, 512], F32, tag="recip")
                nc.vector.tensor_copy(rA[64:65, :], avt_a[64:65, :])
                r0A = ST.tile([1, 512], F32, tag="r0")
                nc.gpsimd.dma_start(r0A[0:1, :], rA[64:65, :])
                denbA = ST.tile([64, 512], F32, tag="denb")
                nc.gpsimd.partition_broadcast(denbA[:], r0A[0:1, :])
                rdenbA = ST.tile([64, 512], F32, tag="rdenb")
                nc.vector.reciprocal_approx_fast(rdenbA[:], denbA[:])
                if dbg:
                    nc.sync.dma_start(dbg_denb[:, 0:512], rdenbA[:])
                nc.vector.tensor_mul(avT2[0:64, hp, c0:c0 + 512],
                                     avt_a[0:64, :], rdenbA[:])
                # head B: normalize at partitions 0-63, DMA-shift to 64-127
                rB = ST.tile([65, 512], F32, tag="recip")
                nc.vector.tensor_copy(rB[64:65, :], avt_b[64:65, :])
                r0B = ST.tile([1, 512], F32, tag="r0")
                nc.gpsimd.dma_start(r0B[0:1, :], rB[64:65, :])
                denbB = ST.tile([64, 512], F32, tag="denb")
                nc.gpsimd.partition_broadcast(denbB[:], r0B[0:1, :])
                rdenbB = ST.tile([64, 512], F32, tag="rdenb")
                nc.vector.reciprocal_approx_fast(rdenbB[:], denbB[:])
                if dbg:
                    nc.sync.dma_start(dbg_denb[:, 512:1024], rdenbB[:])
                tmpB = ST.tile([64, 512], BF16, tag="tmpb")
                nc.vector.tensor_mul(tmpB[:], avt_b[0:64, :], rdenbB[:])
                nc.gpsimd.dma_start(avT2[64:128, hp, c0:c0 + 512], tmpB[:])

            DVE_E, DVE_O = frozenset((2, 6, 10, 14)), frozenset((0, 4, 8, 12))
            for sp in range(2):
                prs = (2 * sp, 2 * sp + 1)
                fillers = (pair_proj_units(2) + pair_proj_units(3)
                           if sp == 0 else [])
                for ich in range(2):
                    c0 = ich * 512
                    avt = {
                        prs[0]: (PSAE.tile([65, 512], F32, tag="avta",
                                           name="avtae"),
                                 PSBE.tile([65, 512], F32, tag="avtb",
                                           name="avtbe")),
                        prs[1]: (PSAO.tile([65, 512], F32, tag="avta",
                                           name="avtao"),
                                 PSBO.tile([65, 512], F32, tag="avtb",
                                           name="avtbo")),
                    }
                    for mt in range(MT + 1):
                        for pr in prs:
                            hA, hB = 2 * pr, 2 * pr + 1
                            if mt < MT:
                                lhsA = kT2[0:64, pr, mt * 128:(mt + 1) * 128]
                                lhsB = kT2[64:128, pr,
                                           mt * 128:(mt + 1) * 128]
                                vA = v65[:, mt, hA, :]
                                vB = v65[:, mt, hB, :]
                            else:
                                lhsA = kT_null2[0:64, :]
                                lhsB = kT_null2[64:128, :]
                                vA = vB = v65_null[:]
                            pool = PSE if pr % 2 == 0 else PSO
                            ps = pool.tile([128, 1024], F32, tag="ps")
                            nc.tensor.matmul(ps[:, 0:512], lhsT=lhsA,
                                             rhs=qT2[0:64, pr, c0:c0 + 512],
                                             start=True, stop=True)
                            nc.tensor.matmul(ps[:, 512:1024], lhsT=lhsB,
                                             rhs=qT2[64:128, pr,
                                                     c0:c0 + 512],
                                             start=True, stop=True)
                            ex = EX.tile([128, 1024], BF16, tag="ex")
                            dve = (mt in DVE_E if pr % 2 == 0
                                   else mt in DVE_O)
                            if dve:
                                scr = SCR.tile([128, 1024], F32, tag="scr")
                                _emit_dve_exp(nc, ex[:], ps[:], scr[:],
                                              cc[:], SCALE)
                            else:
                                nc.scalar.activation(ex[:], ps[:], Act.Exp,
                                                     scale=SCALE)
                            if debug and pr == 0 and ich == 0 and mt < 2:
                                nc.sync.dma_start(
                                    dbg_ex[:, mt * 1024:(mt + 1) * 1024],
                                    ex[:])
                            avt_a, avt_b = avt[pr]
                            nc.tensor.matmul(avt_a[:], lhsT=vA,
                                             rhs=ex[:, 0:512],
                                             start=(mt == 0), stop=(mt == MT))
                            nc.tensor.matmul(avt_b[:], lhsT=vB,
                                             rhs=ex[:, 512:1024],
                                             start=(mt == 0), stop=(mt == MT))
                        if mt in (4, 9, 14) and fillers:
                            fillers.pop(0)()
                    if debug and sp == 0 and ich == 0:
                        avst = ST.tile([65, 1024], F32, tag="avst")
                        nc.vector.tensor_copy(avst[:, 0:512], avt[0][0][:])
                        nc.vector.tensor_copy(avst[:, 512:1024], avt[0][1][:])
                        nc.sync.dma_start(dbg_avt[:], avst[:])
                    for pr in prs:
                        avt_a, avt_b = avt[pr]
                        norm_pair(pr, c0, avt_a, avt_b,
                                  debug and pr == 0 and ich == 0)

            if debug:
                nc.sync.dma_start(
                    dbg_q[:], qT2[:].rearrange("p a b -> p (a b)"))
                nc.sync.dma_start(
                    dbg_k[:], kT2[:].rearrange("p a b -> p (a b)"))
                nc.sync.dma_start(
                    dbg_v[:], v65[:].rearrange("p a b c -> p (a b c)"))
                nc.sync.dma_start(
                    dbg_av[:], avT2[:].rearrange("p a b -> p (a b)"))

            # ---- out-projection at contraction 128 ----
            outT_d = out_d.rearrange("(co p) i -> p co i", p=128)
            for ct in range(4):
                ps_o = (PSE, PSO)[ct % 2].tile([128, 1024], F32, tag="ps")
                for ich in range(2):
                    for hp in range(HP):
                        nc.tensor.matmul(
                            ps_o[:, ich * 512:(ich + 1) * 512],
                            lhsT=wout_b[:, hp, ct * 128:(ct + 1) * 128],
                            rhs=avT2[:, hp, ich * 512:(ich + 1) * 512],
                            start=(hp == 0), stop=(hp == HP - 1))
                ost = ST.tile([128, 1024], F32, tag="ost")
                nc.vector.tensor_add(ost[:], ps_o[:],
                                     bout_sb[:, ct:ct + 1]
                                     .to_broadcast((128, 1024)))
                deng = nc.sync if ct % 2 == 0 else nc.scalar
                deng.dma_start(outT_d[:, ct, :], ost[:])

    nc.compile()
    return nc


def _get_compiled():
    if "nc" not in _COMPILED:
        _COMPILED["nc"] = _build()
    return _COMPILED["nc"]


def kernel(x, context, Wq, Wkv, null_k, null_v, Wout, bout):
    global LAST_EXEC_TIME_NS
    import ml_dtypes
    from concourse.bass_utils import run_bass_kernel_spmd

    BF = ml_dtypes.bfloat16
    x = np.asarray(x, dtype=np.float32)
    context = np.asarray(context, dtype=np.float32)
    nk2 = np.tile(np.asarray(null_k, np.float32).reshape(64, 1), (2, 1)).copy()
    nv = np.asarray(null_v, np.float32).reshape(1, 64)
    bout_r = np.asarray(bout, np.float32).reshape(4, 128).T.copy()
    wq = np.ascontiguousarray(np.asarray(Wq, np.float32)).astype(BF)
    wkv = np.ascontiguousarray(np.asarray(Wkv, np.float32)).astype(BF)
    wout = np.ascontiguousarray(np.asarray(Wout, np.float32)).astype(BF)

    in_maps = []
    ctxT_all = [np.ascontiguousarray(context[b].T).astype(BF)
                for b in range(B)]
    for c in range(N_CORES):
        b, j = c // 2, c % 2
        in_maps.append({
            "x": np.ascontiguousarray(
                x[b, j * NSH:(j + 1) * NSH, :].T).astype(BF),
            "ctx": ctxT_all[b],
            "wq": wq,
            "wkv": wkv,
            "nullk": nk2,
            "nullv": nv,
            "wout": wout,
            "bout": bout_r,
        })

    nc = _get_compiled()
    res = run_bass_kernel_spmd(nc, in_maps, core_ids=list(range(N_CORES)))
    LAST_EXEC_TIME_NS = res.exec_time_ns

    out = np.empty((B, N, DIM), np.float32)
    for c in range(N_CORES):
        b, j = c // 2, c % 2
        out[b, j * NSH:(j + 1) * NSH, :] = res.results[c]["out"].T
    return out

